# revision 1
# baseline (speedup 1.0000x reference)
"""Trainium2 Bass kernel for nn_Blocks2Matrix (scatter_memory).

Strategy (all index math is resolved at trace time, so the scatter becomes a
fully static schedule):
 - Shard systems across the 8 cores (2 systems/core); bucket pair entries by
   (system, row atom). Each pair contributes a direct entry (row=i, col=j)
   and a transposed entry (row=j, col=i, V^T).
 - Device layout ("K-layout"): for each system the [2560, 2560] output is a
   row/col permutation of 25 planes [512, 512] indexed (m1, m2); every entry
   contribution is then matmul-native (8x8 radial tiles).
 - Scatter stage: per slab (sys_local, row atom i), chunks of 16 entries are
   accumulated into PSUM via one-hot matmuls:
       acc[0:40 | 40:80, c] += VA_chunk[128, 80].T @ onehot[128, span]
   VA columns 0:40 hold direct entries' V (radial-major, mu-minor), columns
   40:80 hold transposed entries' V^T; onehot rows are (entry, q) with the
   single 1.0 at col j*8+q. Entries are sorted by col, so non-first chunks
   only touch a narrow column span (first chunk is full-width to zero PSUM).
 - Dense stage: per (slab, pg in {0,1}) one matmul
       pout[(pp, ab) = 100, c] = BD[128, 100].T @ slab_sb[128, 512]
   with BD block-diagonal cg rows 0:40 (direct) and cg^T rows 40:80.
 - fp16 operands (exact for one-hots/iota; ~5e-4 rel on values), fp32 PSUM
   accumulation, fp16 output; host permutes to the reference layout.
"""
import contextlib

import numpy as np

import concourse.bass as bass
import concourse.mybir as mybir
from concourse.bass_utils import run_bass_kernel_spmd

N_SYS, N_ATOMS, NRAD, MU, M1, M2 = 16, 64, 8, 5, 5, 5
S = 32768
NORB = NRAD * M1            # 40
NORB2 = 2 * NORB            # 80 (dir + tra column blocks)
N = N_ATOMS * NORB          # 2560
N_CORES = 8
SYS_PER_CORE = N_SYS // N_CORES
NK = N_ATOMS * NRAD         # 512
CHUNK = 16                  # entries per scatter matmul (K = 128)
N_SLABS = SYS_PER_CORE * N_ATOMS   # 128 per core
KDIM = 128                  # dense-stage contraction rows (padded for FWL)
F32 = mybir.dt.float32
FP16 = mybir.dt.float16


def _preprocess(values, sys_idx, i_idx, j_idx):
    """Build per-core SBUF images.

    Returns (va_img [8,128,nchunk*80] f32, jcol [8,128,nchunk] f32,
             Ck [N_SLABS], spans [nchunk,2]).
    """
    vals = values.reshape(S, MU, NRAD, NRAD)
    sys_idx = np.asarray(sys_idx, dtype=np.int64)
    i_idx = np.asarray(i_idx, dtype=np.int64)
    j_idx = np.asarray(j_idx, dtype=np.int64)

    ent_sys = np.concatenate([sys_idx, sys_idx])
    ent_row = np.concatenate([i_idx, j_idx])
    ent_col = np.concatenate([j_idx, i_idx])
    ent_typ = np.concatenate([np.zeros(S, np.int64), np.ones(S, np.int64)])

    core_of = ent_sys // SYS_PER_CORE
    slab_of = (ent_sys % SYS_PER_CORE) * N_ATOMS + ent_row

    flat = core_of * N_SLABS + slab_of
    counts = np.bincount(flat, minlength=N_CORES * N_SLABS).reshape(N_CORES, N_SLABS)
    Ck = np.maximum(1, -(-counts // CHUNK)).max(axis=0)      # [N_SLABS] joint chunks
    E_slab = Ck * CHUNK
    E_tot = int(E_slab.sum())
    grp_base = np.concatenate([[0], np.cumsum(E_slab)[:-1]])

    order = np.lexsort((ent_col, ent_row, ent_sys))
    oc = core_of[order]
    ogrp = slab_of[order]
    key = oc * N_SLABS + ogrp
    first = np.r_[True, key[1:] != key[:-1]]
    idx = np.arange(key.size)
    start_of_group = np.maximum.accumulate(np.where(first, idx, 0))
    rank = idx - start_of_group
    dest = grp_base[ogrp] + rank

    # VA[e, q, 0:40]  = V[mu, p, q]   (p*5+mu, direct entries)
    # VA[e, q, 40:80] = V[mu, q', p'] (transposed entries)
    VA_dir = vals.transpose(0, 3, 2, 1).reshape(S, NRAD, NORB)
    VA_tra = vals.transpose(0, 2, 3, 1).reshape(S, NRAD, NORB)
    ent_VA = np.zeros((2 * S, NRAD, NORB2), dtype=np.float32)
    ent_VA[:S, :, :NORB] = VA_dir
    ent_VA[S:, :, NORB:] = VA_tra

    va = np.zeros((N_CORES, E_tot, NRAD, NORB2), dtype=np.float32)
    va[oc, dest] = ent_VA[order]
    jq = np.zeros((N_CORES, E_tot, NRAD), dtype=np.float32)
    jq[oc, dest] = (ent_col[order, None] * NRAD + np.arange(NRAD)[None, :]).astype(np.float32)

    va_flat = va.reshape(N_CORES, E_tot * NRAD, NORB2)
    nchunk = E_tot * NRAD // 128
    va_img = va_flat.reshape(N_CORES, nchunk, 128, NORB2).transpose(0, 2, 1, 3) \
                    .reshape(N_CORES, 128, nchunk * NORB2).copy()
    jcol = jq.reshape(N_CORES, nchunk, 128).transpose(0, 2, 1).copy()

    # per-chunk one-hot column spans (union over cores; entries sorted by col
    # within each slab). Padding entries (jc=0) never match sliced chunks.
    chunk_of = (dest // CHUNK).astype(np.int64)
    cmin = np.full(nchunk, NK, np.int64)
    cmax = np.full(nchunk, -1, np.int64)
    ecol = ent_col[order] * NRAD
    np.minimum.at(cmin, chunk_of, ecol)
    np.maximum.at(cmax, chunk_of, ecol + NRAD)
    empty = cmax < 0
    cmin[empty], cmax[empty] = 0, NRAD
    spans = np.stack([(cmin // 4) * 4, np.minimum(NK, -(-cmax // 4) * 4)], axis=1)
    return va_img, jcol, Ck, spans


UPPER = [(a, b) for a in range(M1) for b in range(a, M2)]   # 15 (a<=b) pairs
MOUT = NRAD * len(UPPER)                                     # 120 output rows


def _make_bd(cg):
    """BD [128, 120]: H is symmetric, so only the 15 upper (a<=b) cg planes
    are computed; rows 0:40 direct cg[a,b], rows 40:80 transposed cg[b,a],
    rows 80:128 zero (pads K to 128 for fast weight load). Col = p*15+u."""
    bd = np.zeros((KDIM, MOUT), dtype=np.float32)
    for p in range(NRAD):
        for u, (a, b) in enumerate(UPPER):
            for mu in range(MU):
                bd[p * 5 + mu, p * 15 + u] = cg[a, b, mu]
                bd[NORB + p * 5 + mu, p * 15 + u] = cg[b, a, mu]
    return bd


def _postprocess(outs):
    """outs: [8][120, N_SLABS*NK] f32; rows (p,u), cols (sl,i,j,q)."""
    O = np.stack(outs).reshape(N_CORES, NRAD, len(UPPER),
                               SYS_PER_CORE, N_ATOMS, N_ATOMS, NRAD)
    O = O.reshape(N_SYS * 0 + N_CORES * 1, NRAD, len(UPPER), SYS_PER_CORE,
                  N_ATOMS, N_ATOMS, NRAD) if False else O
    # Kfull[sys, a, b, i, p, j, q]; lower planes are transposes of upper
    Kfull = np.empty((N_CORES, SYS_PER_CORE, M1, M2, N_ATOMS, NRAD, N_ATOMS, NRAD),
                     dtype=np.float32)
    for u, (a, b) in enumerate(UPPER):
        # O dims: [core, p, u, sl, i, j, q] -> [core, sl, i, p, j, q]
        plane = O[:, :, u].transpose(0, 2, 3, 1, 4, 5)
        Kfull[:, :, a, b] = plane
        if a != b:
            Kfull[:, :, b, a] = plane.transpose(0, 1, 4, 5, 2, 3)
    # -> H[sys, (i, p, a), (j, q, b)]
    return np.ascontiguousarray(
        Kfull.reshape(N_SYS, M1, M2, N_ATOMS, NRAD, N_ATOMS, NRAD)
             .transpose(0, 3, 4, 1, 5, 6, 2)
    ).reshape(N_SYS, N, N)


def _build_program(Ck, nchunk, spans):
    """Raw-bass SPMD program (explicit semaphores).

    Pipeline per chunk k / slab s:
      DVE : one-hot[k%8] = is_equal(iota, jc[:, k])            -> oh_sem
      PE  : acc[s%4] += va[k].T @ oh[k%8]   (PSUM accumulate)  -> mm_sem
      ACT/DVE (alternating s): slab_sb[s%4][0:80] <- acc copy  -> cpA/cpD_sem
      PE  : pout[s%2][pg] = BD[pg].T @ slab_sb[s%4]            -> ds_sem
      ACT : stage[s%4] <- pout[s%2]  (fp32 -> fp16)            -> stg_sem
      SYNC: DRAM out rows [s*200, (s+1)*200) <- stage[s%4]     -> out_sems[s%4]
    """
    nc = bass.Bass()
    W = nchunk * NORB2

    va_d = nc.declare_dram_parameter("va", [128, W], FP16, isOutput=False)
    io_d = nc.declare_dram_parameter("iota", [128, NK], FP16, isOutput=False)
    jc_d = nc.declare_dram_parameter("jcol", [128, nchunk], F32, isOutput=False)
    bd_d = nc.declare_dram_parameter("bd", [KDIM, MOUT], FP16, isOutput=False)
    out_d = nc.declare_dram_parameter("out", [MOUT, N_SLABS * NK], FP16, isOutput=True)

    # static schedule
    first_k = np.concatenate([[0], np.cumsum(Ck)[:-1]]).astype(int)
    cum_mm = np.cumsum(Ck).astype(int)               # mm_sem value after slab s
    n_chunks = int(cum_mm[-1])
    kspan = spans.copy()
    for s in range(N_SLABS):
        kspan[first_k[s]] = (0, NK)

    # slab-copy engine assignment + per-engine 1-based completion index
    cp_eng = ['D' if s % 5 == 4 else 'A' for s in range(N_SLABS)]
    cp_idx = np.zeros(N_SLABS, dtype=int)
    ca = cd = 0
    for s in range(N_SLABS):
        if cp_eng[s] == 'A':
            ca += 1; cp_idx[s] = ca
        else:
            cd += 1; cp_idx[s] = cd

    # va is loaded in 16 column-stripes with individual semaphores
    n_va_tiles = 16
    va_bnd = [W * t // n_va_tiles for t in range(n_va_tiles + 1)]
    va_bnd = [b - b % NORB2 for b in va_bnd[:-1]] + [W]   # chunk-aligned

    with (
        nc.sbuf_tensor([128, W], FP16) as va_sb,
        nc.sbuf_tensor([128, nchunk], F32) as jc_sb,
        nc.sbuf_tensor([KDIM, MOUT], FP16) as bd_sb,
        nc.sbuf_tensor([128, NK], FP16) as iota_sb,
        nc.sbuf_tensor([128, 8 * NK], FP16) as oh_sb,
        nc.sbuf_tensor([KDIM, 4 * NK], FP16) as slab_sb,
        nc.sbuf_tensor([MOUT, 8 * NK], FP16) as stage_sb,
        nc.psum_tensor([NORB2, 4 * NK], F32) as acc_ps,
        nc.psum_tensor([MOUT, 4 * NK], F32) as pout_ps,
        nc.semaphore("cst_sem") as cst_sem,
        nc.semaphore("oh_sem") as oh_sem,
        nc.semaphore("mm_sem") as mm_sem,
        nc.semaphore("ds_sem") as ds_sem,
        nc.semaphore("cpA_sem") as cpA_sem,
        nc.semaphore("cpD_sem") as cpD_sem,
        nc.semaphore("stg_sem") as stg_sem,
        nc.Block() as block,
    ):
        with contextlib.ExitStack() as stk:
            va_sems = [stk.enter_context(nc.semaphore(f"va_sem{t}"))
                       for t in range(n_va_tiles)]
            out_sems = [stk.enter_context(nc.semaphore(f"out_sem{i}"))
                        for i in range(8)]
            cp_sems = {'A': cpA_sem, 'D': cpD_sem}

            def oh_slice(k, c0, c1):
                base = (k % 8) * NK
                return oh_sb[:, base + c0:base + c1]

            def cp_wait(engine, s):
                engine.wait_ge(cp_sems[cp_eng[s]], int(cp_idx[s]))

            @block.sync
            def _(sync):
                # constants first: every engine gates on cst_sem, so these
                # must not queue behind the big va stripes
                sync.dma_start(out=iota_sb[:], in_=io_d[:]).then_inc(cst_sem, 16)
                sync.dma_start(out=jc_sb[:], in_=jc_d[:]).then_inc(cst_sem, 16)
                sync.dma_start(out=bd_sb[:], in_=bd_d[:]).then_inc(cst_sem, 16)
                for t in range(n_va_tiles):
                    sync.dma_start(out=va_sb[:, va_bnd[t]:va_bnd[t + 1]],
                                   in_=va_d[:, va_bnd[t]:va_bnd[t + 1]]).then_inc(va_sems[t], 16)
                for s in range(N_SLABS):
                    sync.wait_ge(stg_sem, s + 1)
                    sync.dma_start(
                        out=out_d[:, s * NK:(s + 1) * NK],
                        in_=stage_sb[:, (s % 8) * NK:((s % 8) + 1) * NK],
                    ).then_inc(out_sems[s % 8], 16)

            @block.vector
            def _(vector):
                vector.wait_ge(cst_sem, 48)

                def dve_copy(s):
                    vector.wait_ge(mm_sem, int(cum_mm[s]))
                    if s >= 4:
                        vector.wait_ge(ds_sem, s - 3)
                    nc.vector.tensor_copy(
                        out=slab_sb[0:NORB2, (s % 4) * NK:(s % 4 + 1) * NK],
                        in_=acc_ps[:, (s % 4) * NK:(s % 4 + 1) * NK],
                    ).then_inc(cpD_sem, 1)

                k = 0
                for s in range(N_SLABS):
                    for kk in range(int(Ck[s])):
                        c0, c1 = int(kspan[k][0]), int(kspan[k][1])
                        if k >= 8:
                            vector.wait_ge(mm_sem, k - 7)
                        nc.vector.tensor_scalar(
                            out=oh_slice(k, c0, c1), in0=iota_sb[:, c0:c1],
                            scalar1=jc_sb[:, k:k + 1], scalar2=None,
                            op0=mybir.AluOpType.is_equal).then_inc(oh_sem, 1)
                        k += 1
                    if s >= 1 and cp_eng[s - 1] == 'D':
                        dve_copy(s - 1)
                if cp_eng[N_SLABS - 1] == 'D':
                    dve_copy(N_SLABS - 1)

            @block.tensor
            def _(tensor):
                tensor.wait_ge(cst_sem, 48)

                def dense(s):
                    cp_wait(tensor, s)
                    if s >= 4:
                        tensor.wait_ge(stg_sem, s - 3)
                    nc.tensor.matmul(
                        pout_ps[:, (s % 4) * NK:(s % 4 + 1) * NK],
                        bd_sb[:, :],
                        slab_sb[:, (s % 4) * NK:(s % 4 + 1) * NK],
                        start=True, stop=True).then_inc(ds_sem, 1)

                cur_tile = -1
                k = 0
                for s in range(N_SLABS):
                    nck = int(Ck[s])
                    for kk in range(nck):
                        tt = 0
                        while va_bnd[tt + 1] <= k * NORB2:
                            tt += 1
                        if tt != cur_tile:
                            tensor.wait_ge(va_sems[tt], 16)
                            cur_tile = tt
                        tensor.wait_ge(oh_sem, k + 1)
                        if kk == 0 and s >= 4:
                            cp_wait(tensor, s - 4)
                        c0, c1 = int(kspan[k][0]), int(kspan[k][1])
                        nc.tensor.matmul(
                            acc_ps[:, (s % 4) * NK + c0:(s % 4) * NK + c1],
                            va_sb[:, k * NORB2:(k + 1) * NORB2],
                            oh_slice(k, c0, c1),
                            start=(kk == 0), stop=(kk == nck - 1),
                            skip_group_check=True).then_inc(mm_sem, 1)
                        k += 1
                    if s >= 1:
                        dense(s - 1)
                dense(N_SLABS - 1)

            @block.scalar
            def _(scalar):
                scalar.wait_ge(cst_sem, 48)
                # zero rows 64:128 of each slab slot once; copies rewrite
                # 64:80 every slab, rows 80:128 stay zero (K padding)
                for sl in range(4):
                    nc.scalar.activation(
                        out=slab_sb[64:128, sl * NK:(sl + 1) * NK],
                        in_=iota_sb[64:128, :],
                        func=mybir.ActivationFunctionType.Copy, scale=0.0)

                def act_copy(s):
                    scalar.wait_ge(mm_sem, int(cum_mm[s]))
                    if s >= 4:
                        scalar.wait_ge(ds_sem, s - 3)
                    nc.scalar.copy(
                        out=slab_sb[0:NORB2, (s % 4) * NK:(s % 4 + 1) * NK],
                        in_=acc_ps[:, (s % 4) * NK:(s % 4 + 1) * NK],
                    ).then_inc(cpA_sem, 1)

                def stage(s):
                    scalar.wait_ge(ds_sem, s + 1)
                    if s >= 8:
                        scalar.wait_ge(out_sems[s % 8], 16 * (s // 8))
                    nc.scalar.copy(
                        out=stage_sb[:, (s % 8) * NK:((s % 8) + 1) * NK],
                        in_=pout_ps[:, (s % 4) * NK:((s % 4) + 1) * NK],
                    ).then_inc(stg_sem, 1)

                for s in range(N_SLABS):
                    if cp_eng[s] == 'A':
                        act_copy(s)
                    if s >= 1:
                        stage(s - 1)
                stage(N_SLABS - 1)

    return nc


def _run(values, cg, sys_idx, i_idx, j_idx, trace=False):
    import ml_dtypes
    f16 = np.float16
    va_img, jcol, Ck, spans = _preprocess(values, sys_idx, i_idx, j_idx)
    bd = _make_bd(np.asarray(cg, dtype=np.float32)).astype(f16)
    nchunk = int(Ck.sum()) * CHUNK * NRAD // 128
    nc = _build_program(Ck, nchunk, spans)
    iota = np.arange(NK, dtype=np.float16)[None, :].repeat(128, axis=0)
    in_maps = [{"va": va_img[c].astype(f16), "jcol": jcol[c], "bd": bd,
                "iota": iota}
               for c in range(N_CORES)]
    res = run_bass_kernel_spmd(nc, in_maps, list(range(N_CORES)), trace=trace)
    outs = [np.asarray(res.results[c]["out"], dtype=np.float32)
            for c in range(N_CORES)]
    return _postprocess(outs), res


def kernel(values, cg, sys_idx, i_idx, j_idx):
    H, _ = _run(np.asarray(values, dtype=np.float32), cg, sys_idx, i_idx, j_idx)
    return H

